# revision 4
# baseline (speedup 1.0000x reference)
"""Trainium2 Bass kernel for dynamic int8-quantized linear layer.

Reference computation (per nn_CustomLinear):
  - per-row symmetric int8 quantization of weight [O, D]
  - dynamic per-row symmetric int8 quantization of x [B, N, D]
  - int8 GEMM accumulated in int32
  - dequantize with x_scale (per row) * w_scale (per out channel) + bias

Strategy (v2):
  - Data-parallel over 8 NeuronCores: x flattened to [B*N, D] and split in 8
    row shards; weight + bias replicated on every core.
  - Scales are folded into the quantized operands: xqs = fp16(round(x/xs)*xs)
    and wqs = fp16(round(w/ws)*ws).  The fp16 GEMM with fp32 PSUM
    accumulation then directly yields y - bias (products of the two fp16
    casts are exact in fp32; only the casts themselves add ~1e-4 relative
    error, far below the 2e-2 gate).  Dequant collapses to one
    tensor_tensor add of the broadcast bias that also drains PSUM to SBUF.
  - Quantized operands round-trip through DRAM in fp16 and are transposed
    with X-bar DMAs so the contraction dim lands on the partition axis.
  - Engine separation to avoid in-order queue head-of-line blocking:
      sync    : x/w input loads only (never waits on compute)
      DVE     : x stats + x fold, then per-GEMM-round bias-add drains
      ACT     : rounding (x and w), xq writes, all transpose DMAs, y stores
      GpSimd  : w stats + w fold + wq writes (all pre-GEMM)
      PE      : matmuls only
  - Software pipelining: W is processed in 2 leading rounds (oc-groups 0,1
    then 2,3), x in 4 chunk rounds; GEMM work for a (chunk, oc-group) pair
    is issued in the round where both become available, and PSUM drains are
    issued with a one-round lag so they never block pre-GEMM work.
"""

import numpy as np

import concourse.bass as bass
import concourse.mybir as mybir
import concourse.tile as tile
from concourse import bacc
from concourse.bass_utils import run_bass_kernel_spmd

F32 = mybir.dt.float32
F16 = mybir.dt.float16

RND = 12582912.0  # 1.5 * 2**23: fp32 add/sub rounds to nearest int (RNE)
QMAX = 127.0

# Problem shapes (hardcoded; harness calls kernel() with exactly these).
B, N, D, O = 4, 4096, 2048, 2048
N_CORES = 8
P = 128


def build_nc(n_rows=B * N // N_CORES, d=D, o=O, n_cores=N_CORES):
    """Build the single-core Bass program (SPMD: same program on all cores)."""
    nc = bacc.Bacc(
        "TRN2",
        target_bir_lowering=False,
        debug=False,
        num_devices=n_cores,
    )
    x_d = nc.dram_tensor("x", [n_rows, d], F32, kind="ExternalInput").ap()
    w_d = nc.dram_tensor("w", [o, d], F32, kind="ExternalInput").ap()
    b_d = nc.dram_tensor("b", [o], F32, kind="ExternalInput").ap()
    y_d = nc.dram_tensor("y", [n_rows, o], F32, kind="ExternalOutput").ap()

    n_dd = d // P  # contraction tiles
    OC = min(512, o)  # output-column chunk (one PSUM bank)
    n_oc = o // OC
    wg_t = OC // P  # w tiles per oc-group
    CH = min(512, n_rows)  # x rows per transpose chunk
    n_ch = n_rows // CH
    tpc = CH // P  # x tiles per chunk
    n_xt = n_rows // P
    n_wt = o // P

    # W oc-group g is processed in round g // 2 (front-loaded, 2 groups per
    # round) so that every x chunk c >= 1 has all of W available in its own
    # round.  Chunk c's GEMMs then all run by round max(c, 1), which keeps
    # the xqsT buffer reuse distance <= 3 rounds.
    def w_round(g):
        return g // 2

    n_rounds = max(n_ch, (n_oc + 1) // 2)

    with tile.TileContext(nc) as tc:
        with (
            tc.tile_pool(name="consts", bufs=1) as consts,
            tc.tile_pool(name="wqsT_pool", bufs=1) as wqsT_pool,
            tc.tile_pool(name="xqsT_pool", bufs=min(3, n_ch)) as xqsT_pool,
            tc.tile_pool(name="xin", bufs=2) as xin,
            tc.tile_pool(name="win", bufs=2) as win,
            tc.tile_pool(name="xtmp", bufs=2) as xtmp,
            tc.tile_pool(name="wtmp", bufs=1) as wtmp,
            tc.tile_pool(name="xqsp", bufs=2) as xqsp,
            tc.tile_pool(name="wqsp", bufs=2) as wqsp,
            tc.tile_pool(name="ypool", bufs=4) as ypool,
            tc.tile_pool(name="xdram", bufs=2, space="DRAM") as xdram,
            tc.tile_pool(name="wdram", bufs=2, space="DRAM") as wdram,
            tc.tile_pool(name="psum_mm", bufs=8, space="PSUM") as psum_pool,
        ):
            rnd_c = consts.tile([P, 1], F32)
            nc.vector.memset(rnd_c, RND)

            xmax_slab = consts.tile([P, n_xt], F32)
            xs_slab = consts.tile([P, n_xt], F32)
            xrecip_slab = consts.tile([P, n_xt], F32)
            wmax_slab = consts.tile([P, n_wt], F32)
            ws_slab = consts.tile([P, n_wt], F32)
            wrecip_slab = consts.tile([P, n_wt], F32)

            # bias broadcast to all partitions: DRAM [o] -> SBUF [P, o]
            biasb = consts.tile([P, o], F32)
            nc.gpsimd.dma_start(
                out=biasb,
                in_=bass.AP(
                    tensor=b_d.tensor, offset=b_d.offset, ap=[[0, P]] + list(b_d.ap)
                ),
            )

            # quantized*scaled transposed weight, resident: [d_part, dd, o]
            wqsT = wqsT_pool.tile([P, n_dd, o], F16)

            xqsT_tiles = {}
            pending = []  # (psum_tile, x_tile_idx, oc_group) awaiting drain

            def flush_pending():
                for pm, i, g in pending:
                    y_t = ypool.tile([P, OC], F32, name="y_t")
                    nc.vector.tensor_add(y_t, pm, biasb[:, g * OC : (g + 1) * OC])
                    nc.scalar.dma_start(
                        out=y_d[i * P : (i + 1) * P, g * OC : (g + 1) * OC],
                        in_=y_t,
                    )
                pending.clear()

            for r in range(n_rounds):
                # ---------------- X chunk r ----------------
                if r < n_ch:
                    xq_dram = xdram.tile([CH, d], F16, name="xq_dram")
                    for j in range(tpc):
                        i = r * tpc + j
                        x_t = xin.tile([P, d], F32, name="x_t")
                        nc.sync.dma_start(out=x_t, in_=x_d[i * P : (i + 1) * P, :])
                        nc.vector.tensor_reduce(
                            out=xmax_slab[:, i : i + 1],
                            in_=x_t,
                            axis=mybir.AxisListType.X,
                            op=mybir.AluOpType.max,
                            apply_absolute_value=True,
                        )
                        nc.vector.tensor_scalar(
                            out=xs_slab[:, i : i + 1],
                            in0=xmax_slab[:, i : i + 1],
                            scalar1=1.0 / QMAX,
                            scalar2=1e-12,
                            op0=mybir.AluOpType.mult,
                            op1=mybir.AluOpType.max,
                        )
                        nc.vector.reciprocal(
                            out=xrecip_slab[:, i : i + 1], in_=xs_slab[:, i : i + 1]
                        )
                        t1 = xtmp.tile([P, d], F32, name="t1")
                        nc.scalar.activation(
                            out=t1,
                            in_=x_t,
                            func=mybir.ActivationFunctionType.Identity,
                            bias=rnd_c,
                            scale=xrecip_slab[:, i : i + 1],
                        )
                        xqs = xqsp.tile([P, d], F16, name="xqs")
                        nc.vector.tensor_scalar(
                            out=xqs,
                            in0=t1,
                            scalar1=-RND,
                            scalar2=xs_slab[:, i : i + 1],
                            op0=mybir.AluOpType.add,
                            op1=mybir.AluOpType.mult,
                        )
                        nc.scalar.dma_start(
                            out=xq_dram[j * P : (j + 1) * P, :], in_=xqs
                        )
                    xqsT = xqsT_pool.tile([P, n_dd, CH], F16, name="xqsT")
                    for dd in range(n_dd):
                        nc.scalar.dma_start_transpose(
                            out=xqsT[:, dd, :],
                            in_=xq_dram[:, dd * P : (dd + 1) * P],
                        )
                    xqsT_tiles[r] = xqsT

                # ---------------- W groups for this round ----------------
                for g in range(n_oc):
                    if w_round(g) != r:
                        continue
                    wq_dram = wdram.tile([OC, d], F16, name="wq_dram")
                    for j in range(wg_t):
                        t = g * wg_t + j
                        w_t = win.tile([P, d], F32, name="w_t")
                        nc.sync.dma_start(out=w_t, in_=w_d[t * P : (t + 1) * P, :])
                        nc.vector.tensor_reduce(
                            out=wmax_slab[:, t : t + 1],
                            in_=w_t,
                            axis=mybir.AxisListType.X,
                            op=mybir.AluOpType.max,
                            apply_absolute_value=True,
                        )
                        nc.gpsimd.tensor_scalar(
                            out=ws_slab[:, t : t + 1],
                            in0=wmax_slab[:, t : t + 1],
                            scalar1=1e-8,
                            scalar2=1.0 / QMAX,
                            op0=mybir.AluOpType.max,
                            op1=mybir.AluOpType.mult,
                        )
                        nc.vector.reciprocal(
                            out=wrecip_slab[:, t : t + 1], in_=ws_slab[:, t : t + 1]
                        )
                        t1w = wtmp.tile([P, d], F32, name="t1w")
                        nc.scalar.activation(
                            out=t1w,
                            in_=w_t,
                            func=mybir.ActivationFunctionType.Identity,
                            bias=rnd_c,
                            scale=wrecip_slab[:, t : t + 1],
                        )
                        wqs = wqsp.tile([P, d], F16, name="wqs")
                        nc.gpsimd.tensor_scalar(
                            out=wqs,
                            in0=t1w,
                            scalar1=-RND,
                            scalar2=ws_slab[:, t : t + 1],
                            op0=mybir.AluOpType.add,
                            op1=mybir.AluOpType.mult,
                        )
                        nc.gpsimd.dma_start(
                            out=wq_dram[j * P : (j + 1) * P, :], in_=wqs
                        )
                    for dd in range(n_dd):
                        nc.scalar.dma_start_transpose(
                            out=wqsT[:, dd, g * OC : (g + 1) * OC],
                            in_=wq_dram[:, dd * P : (dd + 1) * P],
                        )

                # Drain the previous round's PSUM tiles only after this
                # round's pre-GEMM work is queued (keeps DVE/ACT flowing).
                flush_pending()

                # ---------------- GEMM pairs enabled this round ----------------
                for c in range(min(r + 1, n_ch)):
                    for g in range(n_oc):
                        if max(c, w_round(g)) != r:
                            continue
                        for j in range(tpc):
                            i = c * tpc + j
                            pm = psum_pool.tile([P, OC], F32, name="pm")
                            for dd in range(n_dd):
                                nc.tensor.matmul(
                                    pm,
                                    lhsT=xqsT_tiles[c][:, dd, j * P : (j + 1) * P],
                                    rhs=wqsT[:, dd, g * OC : (g + 1) * OC],
                                    start=(dd == 0),
                                    stop=(dd == n_dd - 1),
                                )
                            pending.append((pm, i, g))

            flush_pending()

    nc.compile()
    return nc


_NC_CACHE = {}


def _get_nc(n_rows, d, o, n_cores):
    key = (n_rows, d, o, n_cores)
    if key not in _NC_CACHE:
        _NC_CACHE[key] = build_nc(n_rows, d, o, n_cores)
    return _NC_CACHE[key]


def kernel(x: np.ndarray, weight: np.ndarray, bias: np.ndarray, **run_kwargs):
    b, n, d = x.shape
    o = weight.shape[0]
    rows = b * n
    n_rows = rows // N_CORES
    nc = _get_nc(n_rows, d, o, N_CORES)

    x_flat = np.ascontiguousarray(np.asarray(x, dtype=np.float32).reshape(rows, d))
    w = np.ascontiguousarray(np.asarray(weight, dtype=np.float32))
    bias = np.ascontiguousarray(np.asarray(bias, dtype=np.float32))

    in_maps = [
        {"x": x_flat[c * n_rows : (c + 1) * n_rows], "w": w, "b": bias}
        for c in range(N_CORES)
    ]
    res = run_bass_kernel_spmd(nc, in_maps, list(range(N_CORES)), **run_kwargs)
    y = np.concatenate([res.results[c]["y"] for c in range(N_CORES)], axis=0)
    out = y.reshape(b, n, o).astype(x.dtype, copy=False)
    if run_kwargs:
        return out, res
    return out


if __name__ == "__main__":
    x = np.random.randn(B, N, D).astype(np.float32)
    w = np.random.randn(O, D).astype(np.float32)
    bias = np.random.randn(O).astype(np.float32)
    y = kernel(x, w, bias)
    print(y.shape, y.dtype)


# revision 6
# speedup vs baseline: 1.0715x; 1.0715x over previous
"""Trainium2 Bass kernel for dynamic int8-quantized linear layer.

Reference computation (per nn_CustomLinear):
  - per-row symmetric int8 quantization of weight [O, D]
  - dynamic per-row symmetric int8 quantization of x [B, N, D]
  - int8 GEMM accumulated in int32
  - dequantize with x_scale (per row) * w_scale (per out channel) + bias

Strategy (v3):
  - Data-parallel over 8 NeuronCores: x flattened to [B*N, D] and split in 8
    row shards; weight + bias replicated on every core.
  - Scales are folded into the quantized operands: xqs = fp16(round(x/xs)*xs)
    and wqs = fp16(round(w/ws)*ws).  The fp16 GEMM with fp32 PSUM
    accumulation then directly yields y - bias (fp16 products are exact in
    fp32; only the two fp16 casts add ~1e-4 relative error, far below the
    2e-2 gate).  Dequant collapses to one tensor_tensor add of the
    broadcast bias that also drains PSUM to SBUF.
  - Quantized operands round-trip through DRAM in fp16 and are transposed
    with X-bar DMAs so the contraction dim lands on the partition axis.
  - Two-phase issue order so the in-order engine queues never head-of-line
    block: phase A issues ALL pre-GEMM work (loads, stats, rounding, folds,
    quantized writes, transposes) for every chunk/group; phase B issues the
    GEMM blocks in data-readiness order, each immediately followed by its
    PSUM drain (bias add) and output store.  Pre-GEMM engines: sync=loads,
    ACT=rounding+transposes, DVE=stats, GpSimd=folds+writes.  Phase B:
    PE=matmuls, DVE=bias drains, ACT=store dispatch.
"""

import numpy as np

import concourse.bass as bass
import concourse.mybir as mybir
import concourse.tile as tile
from concourse import bacc
from concourse.bass_utils import run_bass_kernel_spmd

F32 = mybir.dt.float32
F16 = mybir.dt.float16

RND = 12582912.0  # 1.5 * 2**23: fp32 add/sub rounds to nearest int (RNE)
QMAX = 127.0

# Problem shapes (hardcoded; harness calls kernel() with exactly these).
B, N, D, O = 4, 4096, 2048, 2048
N_CORES = 8
P = 128


def build_nc(n_rows=B * N // N_CORES, d=D, o=O, n_cores=N_CORES):
    """Build the single-core Bass program (SPMD: same program on all cores)."""
    nc = bacc.Bacc(
        "TRN2",
        target_bir_lowering=False,
        debug=False,
        num_devices=n_cores,
    )
    x_d = nc.dram_tensor("x", [n_rows, d], F32, kind="ExternalInput").ap()
    w_d = nc.dram_tensor("w", [o, d], F32, kind="ExternalInput").ap()
    b_d = nc.dram_tensor("b", [o], F32, kind="ExternalInput").ap()
    y_d = nc.dram_tensor("y", [n_rows, o], F32, kind="ExternalOutput").ap()

    n_dd = d // P  # contraction tiles
    OC = min(512, o)  # output-column chunk (one PSUM bank)
    n_oc = o // OC
    wg_t = OC // P  # w tiles per oc-group
    CH = min(512, n_rows)  # x rows per transpose chunk
    n_ch = n_rows // CH
    tpc = CH // P  # x tiles per chunk
    n_xt = n_rows // P
    n_wt = o // P

    with tile.TileContext(nc) as tc:
        with (
            tc.tile_pool(name="consts", bufs=1) as consts,
            tc.tile_pool(name="wqsT_pool", bufs=1) as wqsT_pool,
            tc.tile_pool(name="xqsT_pool", bufs=1) as xqsT_pool,
            tc.tile_pool(name="xin", bufs=2) as xin,
            tc.tile_pool(name="win", bufs=2) as win,
            tc.tile_pool(name="xtmp", bufs=1) as xtmp,
            tc.tile_pool(name="wtmp", bufs=1) as wtmp,
            tc.tile_pool(name="xqsp", bufs=2) as xqsp,
            tc.tile_pool(name="wqsp", bufs=2) as wqsp,
            tc.tile_pool(name="ypool", bufs=3) as ypool,
            tc.tile_pool(name="xdram", bufs=2, space="DRAM") as xdram,
            tc.tile_pool(name="wdram", bufs=2, space="DRAM") as wdram,
            tc.tile_pool(name="psum_mm", bufs=8, space="PSUM") as psum_pool,
        ):
            rnd_c = consts.tile([P, 1], F32)
            nc.vector.memset(rnd_c, RND)

            xmax_slab = consts.tile([P, n_xt], F32)
            xs_slab = consts.tile([P, n_xt], F32)
            xrecip_slab = consts.tile([P, n_xt], F32)
            wmax_slab = consts.tile([P, n_wt], F32)
            ws_slab = consts.tile([P, n_wt], F32)
            wrecip_slab = consts.tile([P, n_wt], F32)

            # bias broadcast to all partitions: DRAM [o] -> SBUF [P, o]
            biasb = consts.tile([P, o], F32)
            nc.gpsimd.dma_start(
                out=biasb,
                in_=bass.AP(
                    tensor=b_d.tensor, offset=b_d.offset, ap=[[0, P]] + list(b_d.ap)
                ),
            )

            # quantized*scaled transposed operands, resident:
            wqsT = wqsT_pool.tile([P, n_dd, o], F16)
            xqsT = xqsT_pool.tile([P, n_dd, n_rows], F16)

            # ---------------- phase A: all pre-GEMM work ----------------
            def w_group(g):
                wq_dram = wdram.tile([OC, d], F16, name="wq_dram")
                for j in range(wg_t):
                    t = g * wg_t + j
                    w_t = win.tile([P, d], F32, name="w_t")
                    nc.sync.dma_start(out=w_t, in_=w_d[t * P : (t + 1) * P, :])
                    nc.vector.tensor_reduce(
                        out=wmax_slab[:, t : t + 1],
                        in_=w_t,
                        axis=mybir.AxisListType.X,
                        op=mybir.AluOpType.max,
                        apply_absolute_value=True,
                    )
                    nc.vector.tensor_scalar(
                        out=ws_slab[:, t : t + 1],
                        in0=wmax_slab[:, t : t + 1],
                        scalar1=1e-8,
                        scalar2=1.0 / QMAX,
                        op0=mybir.AluOpType.max,
                        op1=mybir.AluOpType.mult,
                    )
                    nc.vector.reciprocal(
                        out=wrecip_slab[:, t : t + 1], in_=ws_slab[:, t : t + 1]
                    )
                    t1w = wtmp.tile([P, d], F32, name="t1w")
                    nc.scalar.activation(
                        out=t1w,
                        in_=w_t,
                        func=mybir.ActivationFunctionType.Identity,
                        bias=rnd_c,
                        scale=wrecip_slab[:, t : t + 1],
                    )
                    wqs = wqsp.tile([P, d], F16, name="wqs")
                    nc.gpsimd.tensor_scalar(
                        out=wqs,
                        in0=t1w,
                        scalar1=-RND,
                        scalar2=ws_slab[:, t : t + 1],
                        op0=mybir.AluOpType.add,
                        op1=mybir.AluOpType.mult,
                    )
                    nc.gpsimd.dma_start(
                        out=wq_dram[j * P : (j + 1) * P, :], in_=wqs
                    )
                for dd in range(n_dd):
                    nc.scalar.dma_start_transpose(
                        out=wqsT[:, dd, g * OC : (g + 1) * OC],
                        in_=wq_dram[:, dd * P : (dd + 1) * P],
                    )

            def x_chunk(c):
                xq_dram = xdram.tile([CH, d], F16, name="xq_dram")
                for j in range(tpc):
                    i = c * tpc + j
                    x_t = xin.tile([P, d], F32, name="x_t")
                    nc.sync.dma_start(out=x_t, in_=x_d[i * P : (i + 1) * P, :])
                    nc.vector.tensor_reduce(
                        out=xmax_slab[:, i : i + 1],
                        in_=x_t,
                        axis=mybir.AxisListType.X,
                        op=mybir.AluOpType.max,
                        apply_absolute_value=True,
                    )
                    nc.vector.tensor_scalar(
                        out=xs_slab[:, i : i + 1],
                        in0=xmax_slab[:, i : i + 1],
                        scalar1=1.0 / QMAX,
                        scalar2=1e-12,
                        op0=mybir.AluOpType.mult,
                        op1=mybir.AluOpType.max,
                    )
                    nc.vector.reciprocal(
                        out=xrecip_slab[:, i : i + 1], in_=xs_slab[:, i : i + 1]
                    )
                    t1 = xtmp.tile([P, d], F32, name="t1")
                    nc.scalar.activation(
                        out=t1,
                        in_=x_t,
                        func=mybir.ActivationFunctionType.Identity,
                        bias=rnd_c,
                        scale=xrecip_slab[:, i : i + 1],
                    )
                    xqs = xqsp.tile([P, d], F16, name="xqs")
                    nc.gpsimd.tensor_scalar(
                        out=xqs,
                        in0=t1,
                        scalar1=-RND,
                        scalar2=xs_slab[:, i : i + 1],
                        op0=mybir.AluOpType.add,
                        op1=mybir.AluOpType.mult,
                    )
                    nc.gpsimd.dma_start(
                        out=xq_dram[j * P : (j + 1) * P, :], in_=xqs
                    )
                for dd in range(n_dd):
                    nc.scalar.dma_start_transpose(
                        out=xqsT[:, dd, c * CH : (c + 1) * CH],
                        in_=xq_dram[:, dd * P : (dd + 1) * P],
                    )

            # Issue order: first W group and first x chunk lead so the GEMM
            # stream can start early; the rest alternates W groups and x
            # chunks to keep input DMA in arrival order.
            wg_left = list(range(n_oc))
            ch_left = list(range(n_ch))
            if wg_left:
                w_group(wg_left.pop(0))
            if ch_left:
                x_chunk(ch_left.pop(0))
            while wg_left or ch_left:
                if wg_left:
                    w_group(wg_left.pop(0))
                if ch_left:
                    x_chunk(ch_left.pop(0))

            # ---------------- phase B: GEMM + drains, readiness order --------
            # W group g is issued (g+1)-th among W groups; x chunk c is
            # interleaved after group min(c, n_oc-1).  Block (c, g) becomes
            # ready roughly at issue slot max(c, g); sort accordingly.
            blocks = sorted(
                ((c, g) for c in range(n_ch) for g in range(n_oc)),
                key=lambda cg: (max(cg[0], cg[1]), cg[0], cg[1]),
            )
            for c, g in blocks:
                for j in range(tpc):
                    i = c * tpc + j
                    pm = psum_pool.tile([P, OC], F32, name="pm")
                    for dd in range(n_dd):
                        nc.tensor.matmul(
                            pm,
                            lhsT=xqsT[:, dd, i * P : (i + 1) * P],
                            rhs=wqsT[:, dd, g * OC : (g + 1) * OC],
                            start=(dd == 0),
                            stop=(dd == n_dd - 1),
                        )
                    y_t = ypool.tile([P, OC], F32, name="y_t")
                    nc.vector.tensor_add(y_t, pm, biasb[:, g * OC : (g + 1) * OC])
                    nc.scalar.dma_start(
                        out=y_d[i * P : (i + 1) * P, g * OC : (g + 1) * OC],
                        in_=y_t,
                    )

    nc.compile()
    return nc


_NC_CACHE = {}


def _get_nc(n_rows, d, o, n_cores):
    key = (n_rows, d, o, n_cores)
    if key not in _NC_CACHE:
        _NC_CACHE[key] = build_nc(n_rows, d, o, n_cores)
    return _NC_CACHE[key]


def kernel(x: np.ndarray, weight: np.ndarray, bias: np.ndarray, **run_kwargs):
    b, n, d = x.shape
    o = weight.shape[0]
    rows = b * n
    n_rows = rows // N_CORES
    nc = _get_nc(n_rows, d, o, N_CORES)

    x_flat = np.ascontiguousarray(np.asarray(x, dtype=np.float32).reshape(rows, d))
    w = np.ascontiguousarray(np.asarray(weight, dtype=np.float32))
    bias = np.ascontiguousarray(np.asarray(bias, dtype=np.float32))

    in_maps = [
        {"x": x_flat[c * n_rows : (c + 1) * n_rows], "w": w, "b": bias}
        for c in range(N_CORES)
    ]
    res = run_bass_kernel_spmd(nc, in_maps, list(range(N_CORES)), **run_kwargs)
    y = np.concatenate([res.results[c]["y"] for c in range(N_CORES)], axis=0)
    out = y.reshape(b, n, o).astype(x.dtype, copy=False)
    if run_kwargs:
        return out, res
    return out


if __name__ == "__main__":
    x = np.random.randn(B, N, D).astype(np.float32)
    w = np.random.randn(O, D).astype(np.float32)
    bias = np.random.randn(O).astype(np.float32)
    y = kernel(x, w, bias)
    print(y.shape, y.dtype)
